# revision 1
# baseline (speedup 1.0000x reference)
"""Trainium2 Bass kernel for nn_ExperimentModel (embed -> LN -> S4D -> mean-pool -> linear).

Math: mean-pooling commutes with the causal conv, so
  pooled[b,m] = (1/L) * sum_l u[b,l,m] * W[m,l],
  W[m,l] = WcD[m] - rho[m, L-l],  rho[m,s] = sum_n g[m,n] q_n^s,
  q_n = exp(dt*A_n), g = C/(1-q), WcD = sum_n g[m,n] + D[m].
rho is negligible for s > LTAIL, so for l < L-LTAIL the weight is the
per-channel constant WcD[m] and the bulk contribution reduces to
  (WcD[m]/L) * sum_v cnt_b[v] * enorm[v,m]
where cnt_b is a histogram of the first L-LTAIL tokens. The histogram is
built ON DEVICE as a 2-level one-hot product: v = 64*hi + lo, DVE builds
A[i,hi]=1[x_i>>6==hi] and B[i,lo]=1[(x_i&63)==lo] via is_equal against an
iota tile (bf16, 2x mode), then PE contracts positions chunk-by-chunk:
H[lo,hi] += B_chunk^T A_chunk, reshaped to cnt[v%128, v//128] by two
strided PSUM->SBUF copies. Bulk sums are 16 accumulating PE matmuls
contracting vocab ranks, emitted in [m, b] orientation so no transpose
is needed before the classifier.

Only the last LTAIL=896 positions are gathered (one SBUF-source
transposed dma_gather per batch from the bf16 LN'd table) and reduced
with the materialized tail weight. LN row stats are computed by PE
ones-matmuls from a host-transposed copy of the table (sum and
sum-of-squares of each vocab row land directly in [v%128, rank]
orientation). Batch-parallel over 8 cores (4 batches each).
"""

import numpy as np
import ml_dtypes
from contextlib import ExitStack

import concourse.bass as bass
import concourse.bacc as bacc
import concourse.tile as tile
from concourse import mybir

B, L, V, M, N = 32, 4096, 2000, 128, 64
LTAIL = 384
NBULK = L - LTAIL          # 3712
NCHUNK = NBULK // 128      # 29
NCORES = 8
BPC = B // NCORES
LN_EPS = 1e-5
VPAD = 2048                # vocab rows padded to 16 ranks of 128
NRANK = VPAD // 128

# params column map (f32 [128, PRM_F]); ramp occupies [0:64, RAMP0:RAMP0+LTAIL]
ALOG0 = 0                  # [0:64, 0]         A_log
DCOL0 = 1                  # [:, 1]            D
LOGDT0 = 2                 # [:, 2]            log_dt (as column over m)
WCLS0 = 3                  # [:, 3:5]          W_cls^T
CT0 = 5                    # [0:64, 5:133]     C_re^T
LNW0 = 133                 # [0:1, 133:261]    ln_w row
LNB0 = 261                 # [0:1, 261:389]    ln_b row
BCLS0 = 389                # [0:1, 389:391]    b_cls
PRM_F = 392

f32 = mybir.dt.float32
bf16 = mybir.dt.bfloat16
i16 = mybir.dt.int16
AF = mybir.ActivationFunctionType
OP = mybir.AluOpType


def declare_io(nc):
    embt_d = nc.dram_tensor("emb_t", [128, VPAD], bf16, kind="ExternalInput")
    embl_d = nc.dram_tensor("emb_l", [128, NRANK * 128], bf16, kind="ExternalInput")
    blob_d = nc.dram_tensor("blob_l", [128, PRM_F], f32, kind="ExternalInput")
    ramp_d = nc.dram_tensor("ramp_l", [64, LTAIL], f32, kind="ExternalInput")
    iotacm_d = nc.dram_tensor("iota_cm", [128, 96 * NCHUNK], bf16,
                              kind="ExternalInput")
    xv_d = nc.dram_tensor("xvals", [128, BPC * 2 * NCHUNK], bf16,
                          kind="ExternalInput")
    tidx_d = nc.dram_tensor("tidx", [128, BPC * LTAIL // 16], i16,
                            kind="ExternalInput")
    out_d = nc.dram_tensor("out", [BPC, 2], f32, kind="ExternalOutput")
    return embt_d, embl_d, blob_d, ramp_d, iotacm_d, xv_d, tidx_d, out_d


def emit_body(nc, tc, ctx, io, ln_affine):
    embt_d, embl_d, blob_d, ramp_d, iotacm_d, xv_d, tidx_d, out_d = io
    singles = ctx.enter_context(tc.tile_pool(name="singles", bufs=1))
    work = ctx.enter_context(tc.tile_pool(name="work", bufs=2))
    small = ctx.enter_context(tc.tile_pool(name="small", bufs=1))
    psum = ctx.enter_context(tc.tile_pool(name="psum", bufs=1, space="PSUM"))

    # ---- input DMAs; issue order = landing order on the serial DMA stream
    xv_sb = singles.tile([128, BPC, 2, NCHUNK], bf16)
    nc.sync.dma_start(out=xv_sb, in_=xv_d[:].rearrange(
        "p (b t c) -> p b t c", t=2, c=NCHUNK))
    iotacm_sb = singles.tile([128, 96, NCHUNK], bf16)
    nc.sync.dma_start(out=iotacm_sb, in_=iotacm_d[:].rearrange(
        "p (k c) -> p k c", c=NCHUNK))
    embt_sb = singles.tile([128, VPAD], bf16)
    nc.sync.dma_start(out=embt_sb, in_=embt_d[:])
    embl_sb = singles.tile([128, NRANK, 128], bf16)
    nc.sync.dma_start(out=embl_sb.rearrange("p r m -> p (r m)"), in_=embl_d[:])
    blob = singles.tile([128, PRM_F], f32)
    nc.sync.dma_start(out=blob, in_=blob_d[:])
    ramp_sb = singles.tile([64, LTAIL], f32)
    nc.sync.dma_start(out=ramp_sb, in_=ramp_d[:])
    tidx_sb = singles.tile([128, BPC * LTAIL // 16], i16)
    nc.sync.dma_start(out=tidx_sb, in_=tidx_d[:])

    # ---- S4D weight construction (tiny ops + 3 small matmuls; early deps)
    dt_col = small.tile([128, 1], f32)
    nc.scalar.activation(dt_col, blob[:, LOGDT0:LOGDT0 + 1], AF.Exp)
    expa = small.tile([64, 1], f32)
    nc.scalar.activation(expa, blob[0:64, ALOG0:ALOG0 + 1], AF.Exp)
    ones_1x64 = small.tile([1, 64], f32)
    nc.vector.memset(ones_1x64, 1.0)
    dt0_ps = psum.tile([64, 1], f32, tag="tiny")
    nc.tensor.matmul(dt0_ps, lhsT=ones_1x64, rhs=dt_col[0:1, :], start=True, stop=True)
    c_col = small.tile([64, 1], f32)  # c_n = -exp(A_log_n)*dt  (negative)
    nc.vector.scalar_tensor_tensor(
        out=c_col, in0=expa, scalar=-1.0, in1=dt0_ps, op0=OP.mult, op1=OP.mult)
    q_col = small.tile([64, 1], f32)
    nc.scalar.activation(q_col, c_col, AF.Exp)
    one_col = small.tile([64, 1], f32)
    nc.vector.memset(one_col, 1.0)
    omq = small.tile([64, 1], f32)
    nc.vector.tensor_sub(omq, one_col, q_col)
    wrec = small.tile([64, 1], f32)
    nc.vector.reciprocal(wrec, omq)
    g_sb = singles.tile([64, 128], f32)  # g^T[n,m] = C^T[n,m] / (1 - q_n)
    nc.vector.tensor_scalar_mul(g_sb, blob[0:64, CT0:CT0 + 128], scalar1=wrec)
    p_sb = singles.tile([64, LTAIL], f32)  # q_n^(LTAIL - j)
    nc.scalar.activation(p_sb, ramp_sb, AF.Exp, scale=c_col)
    rho_ps = []
    for h in range(2):
        rp = psum.tile([128, LTAIL // 2], f32, tag=f"rho{h}")
        nc.tensor.matmul(rp, lhsT=g_sb,
                         rhs=p_sb[:, h * (LTAIL // 2):(h + 1) * (LTAIL // 2)],
                         start=True, stop=True)
        rho_ps.append(rp)
    kt_ps = psum.tile([128, 1], f32, tag="tiny")
    nc.tensor.matmul(kt_ps, lhsT=g_sb, rhs=one_col, start=True, stop=True)
    wcd = small.tile([128, 1], f32)  # (Ktot + D) / L
    nc.vector.tensor_add(wcd, kt_ps, blob[:, DCOL0:DCOL0 + 1])
    nc.vector.tensor_scalar_mul(wcd, wcd, scalar1=1.0 / L)
    # tail weight W^T[m, j] for j in [0, LTAIL): (WcD - rho), unscaled
    w_t = singles.tile([128, LTAIL], bf16)
    nc.vector.memset(w_t, 1.0)
    wcd_raw = small.tile([128, 1], f32)
    nc.vector.tensor_add(wcd_raw, kt_ps, blob[:, DCOL0:DCOL0 + 1])
    nc.vector.tensor_scalar_mul(w_t, w_t, scalar1=wcd_raw)
    for h in range(2):
        sl = slice(h * (LTAIL // 2), (h + 1) * (LTAIL // 2))
        nc.vector.scalar_tensor_tensor(
            out=w_t[:, sl], in0=rho_ps[h], scalar=-1.0, in1=w_t[:, sl],
            op0=OP.mult, op1=OP.add)

    # ---- LN stats inputs
    hp0 = tc.high_priority()
    hp0.__enter__()
    sq_t = singles.tile([128, VPAD], bf16)
    nc.scalar.square(sq_t, embt_sb)
    ones_b = small.tile([128, 1], bf16)
    nc.vector.memset(ones_b, 1.0)
    sums_ps = psum.tile([128, NRANK], f32, tag="sums")
    sumsq_ps = psum.tile([128, NRANK], f32, tag="sumsq")
    hp0.__exit__(None, None, None)

    # ---- per-batch: rep (DVE 4x copies), compare (DVE 2x), hist (PE), cnt (ACT)
    # histogram PSUM [lo, hi] -> cntT [v%128, rank, batch] bf16
    # v = 64*hi + lo: v%128 = 64*(hi%2) + lo, v//128 = hi//2
    cnt_t = singles.tile([128, NRANK, BPC], bf16)

    h_ps = {}

    def emit_batch(b):
        xr = work.tile([128, 96, NCHUNK], bf16, tag="xr")
        hiv = xv_sb[:, b, 0, :]
        lov = xv_sb[:, b, 1, :]
        hi_b = bass.AP(hiv.tensor, hiv.offset, [hiv.ap[0], [0, 32], hiv.ap[-1]])
        lo_b = bass.AP(lov.tensor, lov.offset, [lov.ap[0], [0, 64], lov.ap[-1]])
        nc.vector.tensor_copy(xr[:, 0:32, :], hi_b)
        nc.vector.tensor_copy(xr[:, 32:96, :], lo_b)
        ab = work.tile([128, 96, NCHUNK], bf16, tag="ab")
        nc.vector.tensor_tensor(out=ab, in0=xr, in1=iotacm_sb, op=OP.is_equal)
        hp = psum.tile([64, 32], f32, tag="hist")
        for c in range(NCHUNK):
            nc.tensor.matmul(hp, lhsT=ab[:, 32:96, c], rhs=ab[:, 0:32, c],
                             start=(c == 0), stop=(c == NCHUNK - 1))
        h_ps[b] = hp

    def emit_cnt(b):
        hp = h_ps[b]
        nc.scalar.copy(cnt_t[0:64, :, b:b + 1].rearrange("p r o -> p (r o)"),
                       hp[:, 0::2])
        nc.scalar.copy(cnt_t[64:128, :, b:b + 1].rearrange("p r o -> p (r o)"),
                       hp[:, 1::2])

    emit_batch(0)
    emit_batch(1)
    emit_cnt(0)
    emit_cnt(1)

    # ---- LN stats matmuls + mean/rstd chain, slotted between batches 1 and 2
    # so the DVE picks them up before the last two compares.
    hp1 = tc.high_priority()
    hp1.__enter__()
    for r in range(NRANK):
        nc.tensor.matmul(sums_ps[:, r:r + 1],
                         lhsT=embt_sb[:, r * 128:(r + 1) * 128],
                         rhs=ones_b, start=True, stop=True)
        nc.tensor.matmul(sumsq_ps[:, r:r + 1],
                         lhsT=sq_t[:, r * 128:(r + 1) * 128],
                         rhs=ones_b, start=True, stop=True)
    hp1.__exit__(None, None, None)
    hp_ctx = tc.high_priority()
    hp_ctx.__enter__()
    mean = small.tile([128, NRANK], f32)
    nc.vector.tensor_scalar_mul(mean, sums_ps, scalar1=1.0 / 128.0)
    meansq = small.tile([128, NRANK], f32)
    nc.vector.tensor_mul(meansq, mean, mean)
    vpe = small.tile([128, NRANK], f32)  # var + eps
    nc.vector.scalar_tensor_tensor(
        out=vpe, in0=sumsq_ps, scalar=1.0 / 128.0, in1=meansq,
        op0=OP.mult, op1=OP.subtract)
    nc.vector.tensor_single_scalar(vpe, vpe, LN_EPS, OP.add)
    # rstd = rsqrt(var+eps): Quake seed + 2 Newton steps, all on DVE
    shi = small.tile([128, NRANK], mybir.dt.uint32)
    nc.vector.tensor_single_scalar(
        shi, vpe.bitcast(mybir.dt.uint32), 1, OP.logical_shift_right)
    y = small.tile([128, NRANK], mybir.dt.int32)
    nc.vector.tensor_scalar(
        out=y, in0=shi, scalar1=-1.0, scalar2=float(0x5F3759DF),
        op0=OP.mult, op1=OP.add)
    rstd_all = small.tile([128, NRANK], f32)
    yf = y.bitcast(f32)
    t1 = small.tile([128, NRANK], f32)
    cur = yf
    for it in range(2):
        nc.vector.tensor_mul(t1, cur, cur)
        nc.vector.scalar_tensor_tensor(
            out=t1, in0=t1, scalar=-0.5, in1=vpe, op0=OP.mult, op1=OP.mult)
        nc.vector.scalar_tensor_tensor(
            out=rstd_all, in0=t1, scalar=1.5, in1=cur, op0=OP.add, op1=OP.mult)
        cur = rstd_all
    negmurs = small.tile([128, NRANK], f32)
    nc.vector.scalar_tensor_tensor(
        out=negmurs, in0=mean, scalar=-1.0, in1=rstd_all, op0=OP.mult, op1=OP.mult)
    hp_ctx.__exit__(None, None, None)

    enorm = singles.tile([128, NRANK, 128], bf16)
    if ln_affine:
        ones_1x128 = small.tile([1, 128], f32)
        nc.vector.memset(ones_1x128, 1.0)
        lnw_ps = psum.tile([128, 128], f32, tag="sums")
        nc.tensor.matmul(lnw_ps, lhsT=ones_1x128, rhs=blob[0:1, LNW0:LNW0 + 128],
                         start=True, stop=True)
        lnb_ps = psum.tile([128, 128], f32, tag="sumsq")
        nc.tensor.matmul(lnb_ps, lhsT=ones_1x128, rhs=blob[0:1, LNB0:LNB0 + 128],
                         start=True, stop=True)
        lnw_bc = singles.tile([128, 128], f32)
        nc.scalar.copy(lnw_bc, lnw_ps)
        lnb_bc = singles.tile([128, 128], f32)
        nc.scalar.copy(lnb_bc, lnb_ps)
    hp2 = tc.high_priority()
    hp2.__enter__()
    for s in range(NRANK):
        if ln_affine:
            tmp = small.tile([128, 128], f32, tag="lntmp")
            nc.scalar.activation(tmp, embl_sb[:, s, :], AF.Identity,
                                 bias=negmurs[:, s:s + 1], scale=rstd_all[:, s:s + 1])
            nc.vector.tensor_mul(tmp, tmp, lnw_bc)
            nc.vector.tensor_add(enorm[:, s, :], tmp, lnb_bc)
        elif s % 4 == 3:
            nc.vector.tensor_scalar(
                out=enorm[:, s, :], in0=embl_sb[:, s, :],
                scalar1=mean[:, s:s + 1], scalar2=rstd_all[:, s:s + 1],
                op0=OP.subtract, op1=OP.mult)
        else:
            nc.scalar.activation(enorm[:, s, :], embl_sb[:, s, :], AF.Identity,
                                 bias=negmurs[:, s:s + 1], scale=rstd_all[:, s:s + 1])
    # ---- tail: merged transposed SBUF gather (2 calls over 4 batch tails)
    enorm_flat = enorm.rearrange("p r m -> p (r m)")
    pc = small.tile([128, BPC], f32)
    u_all = singles.tile([128, BPC * LTAIL], bf16)
    TCHUNKS = [(0, 768), (768, 768)]
    for gi, (c0, n) in enumerate(TCHUNKS):
        nc.gpsimd.dma_gather(
            out_ap=u_all[:, c0:c0 + n].rearrange("p (c l) -> p c l", c=1),
            in_ap=enorm_flat,
            idxs_ap=tidx_sb[:, c0 // 16:(c0 + n) // 16],
            num_idxs=n,
            num_idxs_reg=n,
            elem_size=128,
            transpose=True,
            sbuf_tokens_per_rank=128,
            sbuf_free_dim_per_rank=256,
            queue_num=gi % 4,
        )
    hp2.__exit__(None, None, None)
    emit_batch(2)
    emit_batch(3)
    emit_cnt(2)
    emit_cnt(3)

    for b in range(BPC):
        prod = work.tile([128, LTAIL], bf16, tag="prod")
        nc.vector.scalar_tensor_tensor(
            out=prod, in0=u_all[:, b * LTAIL:(b + 1) * LTAIL], scalar=1.0 / L,
            in1=w_t, op0=OP.mult, op1=OP.mult, accum_out=pc[:, b:b + 1])

    # ---- bulk sums: 16 accumulating matmuls -> PSUM [m, b]
    bulk_ps = psum.tile([128, BPC], f32, tag="bulk")
    for r in range(NRANK):
        nc.tensor.matmul(bulk_ps, lhsT=enorm[:, r, :], rhs=cnt_t[:, r, :],
                         start=(r == 0), stop=(r == NRANK - 1))

    # ---- pooled[m, b] = bulk_ps * wcd + pc; classifier
    pooled = small.tile([128, BPC], f32)
    nc.vector.scalar_tensor_tensor(
        out=pooled, in0=bulk_ps, scalar=wcd, in1=pc, op0=OP.mult, op1=OP.add)
    ones_1xb = small.tile([1, BPC], f32)
    nc.vector.memset(ones_1xb, 1.0)
    logits_ps = psum.tile([BPC, 2], f32, tag="tiny")
    nc.tensor.matmul(logits_ps, lhsT=pooled, rhs=blob[:, WCLS0:WCLS0 + 2],
                     start=True, stop=False)
    nc.tensor.matmul(logits_ps, lhsT=ones_1xb, rhs=blob[0:1, BCLS0:BCLS0 + 2],
                     start=False, stop=True)
    out_sb = small.tile([BPC, 2], f32)
    nc.vector.tensor_copy(out_sb, logits_ps)
    nc.sync.dma_start(out=out_d[:], in_=out_sb)


def tidx_sb_view(tidx_d):
    return tidx_d[:]


def build_program(ln_affine: bool, repeat: int = 1):
    nc = bacc.Bacc("TRN2", target_bir_lowering=False, debug=False,
                   num_swdge_queues=4)
    io = declare_io(nc)
    with tile.TileContext(nc) as tc:
        with ExitStack() as ctx:
            if repeat == 1:
                emit_body(nc, tc, ctx, io, ln_affine)
            else:
                with tc.For_i(0, repeat, 1):
                    emit_body(nc, tc, ctx, io, ln_affine)
    nc.compile()
    return nc


_PROG_CACHE = {}


def _get_prog(ln_affine: bool, repeat: int = 1):
    key = (ln_affine, repeat)
    if key not in _PROG_CACHE:
        _PROG_CACHE[key] = build_program(ln_affine, repeat)
    return _PROG_CACHE[key]


def host_prep(x, emb, ln_w, ln_b, A_log, D, C_re, log_dt, W_cls, b_cls):
    """Index marshaling + param packing: reshape/transpose/pad/bit-split."""
    x = np.asarray(x)
    assert x.shape == (B, L)
    emb = np.asarray(emb, dtype=np.float32)
    assert np.allclose(log_dt, log_dt[0]), "factorized S4D path needs scalar dt"

    emb_pad = np.zeros((VPAD, M), dtype=np.float32)
    emb_pad[:V] = emb
    emb_t = np.ascontiguousarray(emb_pad.T).astype(ml_dtypes.bfloat16)
    emb_l = np.ascontiguousarray(
        emb_pad.reshape(NRANK, 128, M).transpose(1, 0, 2).reshape(128, NRANK * M)
    ).astype(ml_dtypes.bfloat16)

    blob = np.zeros((128, PRM_F), dtype=np.float32)
    blob[0:64, ALOG0] = np.asarray(A_log, dtype=np.float32)
    blob[:, DCOL0] = np.asarray(D, dtype=np.float32)
    blob[:, LOGDT0] = np.asarray(log_dt, dtype=np.float32)
    blob[:, WCLS0:WCLS0 + 2] = np.asarray(W_cls, dtype=np.float32).T
    blob[0, LNW0:LNW0 + 128] = np.asarray(ln_w, dtype=np.float32)
    blob[0, LNB0:LNB0 + 128] = np.asarray(ln_b, dtype=np.float32)
    blob[0, BCLS0:BCLS0 + 2] = np.asarray(b_cls, dtype=np.float32)
    blob[0:64, CT0:CT0 + 128] = np.asarray(C_re, dtype=np.float32).T
    ramp_l = np.ascontiguousarray(np.tile(
        (LTAIL - np.arange(LTAIL, dtype=np.float32))[None, :], (64, 1)))

    iota96 = np.concatenate(
        [np.arange(32, dtype=np.float32), np.arange(64, dtype=np.float32)])
    iota_cm = np.ascontiguousarray(np.broadcast_to(
        iota96[None, :, None], (128, 96, NCHUNK)).reshape(128, 96 * NCHUNK)
    ).astype(ml_dtypes.bfloat16)

    ln_affine = not (np.all(np.asarray(ln_w) == 1.0) and np.all(np.asarray(ln_b) == 0.0))

    in_maps = []
    for k in range(NCORES):
        xc = x[k * BPC:(k + 1) * BPC].astype(np.int16)              # [BPC, L]
        m = {"emb_t": emb_t, "emb_l": emb_l, "blob_l": blob, "ramp_l": ramp_l,
             "iota_cm": iota_cm}
        xv = np.zeros((128, BPC, 2, NCHUNK), dtype=ml_dtypes.bfloat16)
        for b in range(BPC):
            bulk = xc[b, :NBULK].reshape(NCHUNK, 128).T             # [128, NCHUNK]
            xv[:, b, 0, :] = (bulk >> 6).astype(ml_dtypes.bfloat16)
            xv[:, b, 1, :] = (bulk & 63).astype(ml_dtypes.bfloat16)
        m["xvals"] = np.ascontiguousarray(xv.reshape(128, BPC * 2 * NCHUNK))
        tails = xc[:, NBULK:].reshape(-1)                           # [BPC*LTAIL]
        m["tidx"] = np.ascontiguousarray(np.tile(tails.reshape(-1, 16).T, (8, 1)))
        in_maps.append(m)
    return in_maps, ln_affine


def kernel(**inputs):
    from concourse.bass_utils import run_bass_kernel_spmd

    in_maps, ln_affine = host_prep(**inputs)
    nc = _get_prog(ln_affine)
    res = run_bass_kernel_spmd(nc, in_maps, core_ids=list(range(NCORES)))
    out = np.concatenate([res.results[k]["out"] for k in range(NCORES)], axis=0)
    return out.astype(np.float32)

